# revision 4
# baseline (speedup 1.0000x reference)
"""Cosine attention kernel for Trainium2, sharded over 8 NeuronCores.

Problem: N=4, L=S=2048, H=8, D=64 fp32.
  q = queries / ||queries||_D ; k = keys / ||keys||_D
  qk = einsum('nlhd,nshd->nlsh', q, k); A = softmax(qk / temp, axis=S)
  out = einsum('nlsh,nshd->nlhd', A, values)

Sharding: the 32 (n, h) pairs are split 4-per-core (data + head parallel).
Each core computes 4 independent 2048x2048 attention problems.

v2 design (ACT-bound target):
  - All matmul operands bf16. Row norms: DVE squares+reduce, one ACT Rsqrt
    for all pairs, exp table preloaded right after.
  - QK^T uses PE row-group tiling: knT packed as [128, j, 128] with even
    s-tile's D-rows on partitions 0-63 and odd s-tile's on 64-127; qnT is
    duplicated into both partition halves. Two K=64 matmuls with
    tile_position (0,0)/(64,0) run concurrently on the array.
  - Main loop (per pair, per L-half g, per s-tile-pair j): mm1 e/o for two
    512-col chunks reusing resident weights, one Exp per [128,2,512] PSUM
    tile -> bf16 SBUF, mm2 [128,65]x[128,512] accumulating out^T (ones
    column of V accumulates the softmax denominator in row 64).
  - Epilogue per (pair, g): copy ps2 -> bf16, PE-transpose [65,128] blocks,
    DVE reciprocal + scale, DMA out.
  - Transposes/normalize of pair p+1 are interleaved into main(p)'s issue
    stream so the ACT engine never waits at pair transitions.
"""

import sys

if "/opt/trn_rl_repo" not in sys.path:
    sys.path.insert(0, "/opt/trn_rl_repo")

import numpy as np

N_CORES = 8
PAIRS = 4          # (n, h) pairs per core
L = 2048           # query length
S = 2048           # key length
D = 64             # head dim
T = S // 128       # 128-row s-tiles per pair (16)
TP = T // 2        # s-tile pairs (8)
NG = 2             # L halves
GCH = 2            # 512-col chunks per L half
CW = 512           # chunk width

_PROGRAM_CACHE = {}


def _build_program():
    import concourse.tile as tile
    import concourse.bass as bass
    from concourse import bacc, mybir
    from concourse.bass import ds
    from concourse.masks import make_identity

    f32 = mybir.dt.float32
    bf16 = mybir.dt.bfloat16
    AF = mybir.ActivationFunctionType

    nc = bacc.Bacc("TRN2", target_bir_lowering=False, debug=False,
                   num_devices=N_CORES)
    q_hbm = nc.dram_tensor("q", [PAIRS, L, D], f32, kind="ExternalInput")
    k_hbm = nc.dram_tensor("k", [PAIRS, S, D], f32, kind="ExternalInput")
    v_hbm = nc.dram_tensor("v", [PAIRS, S, D], f32, kind="ExternalInput")
    t_hbm = nc.dram_tensor("temp", [1, 1], f32, kind="ExternalInput")
    o_hbm = nc.dram_tensor("o", [PAIRS, L, D], f32, kind="ExternalOutput")

    with tile.TileContext(nc) as tc:
        with (
            tc.tile_pool(name="const", bufs=1) as cpool,
            tc.tile_pool(name="raw", bufs=1) as raw_pool,
            tc.tile_pool(name="norm", bufs=2) as norm_pool,
            tc.tile_pool(name="big", bufs=2) as big_pool,
            tc.tile_pool(name="io", bufs=2) as io_pool,
            tc.tile_pool(name="work", bufs=2) as work_pool,
            tc.tile_pool(name="small", bufs=4) as small_pool,
            tc.tile_pool(name="pexp", bufs=4) as pexp_pool,
            tc.tile_pool(name="psum1", bufs=2, space="PSUM") as psum1_pool,
            tc.tile_pool(name="psum2", bufs=1, space="PSUM") as psum2_pool,
            tc.tile_pool(name="ptp", bufs=2, space="PSUM") as ptp_pool,
            tc.tile_pool(name="dram", bufs=1, space="DRAM") as dram_pool,
        ):
            identity = cpool.tile([128, 128], f32)
            make_identity(nc, identity[:])
            identity_bf = cpool.tile([128, 128], bf16)
            nc.vector.tensor_copy(identity_bf[:], identity[:])

            scratch_f = cpool.tile([128, 512], f32)
            nc.vector.memset(scratch_f[:], 0.0)
            scratch_bf = cpool.tile([128, 512], bf16)
            nc.vector.tensor_copy(scratch_bf[:], scratch_f[:])

            def warm(n):
                # N=512 bf16 regular matmuls: HAM activity to keep the PE
                # p-state ramped while DMAs stream in.
                for _ in range(n):
                    wk = psum1_pool.tile([128, 2, CW], f32, tag="ps1", name="wk")
                    nc.tensor.matmul(wk[:, 0, :], identity_bf[:], scratch_bf[:])

            warm(10)

            # 1/temp broadcast to [128, 1] (bounce through DRAM for the
            # partition-broadcast DMA).
            t_sb = cpool.tile([1, 1], f32)
            nc.sync.dma_start(t_sb[:], t_hbm.ap())
            rt_sb = cpool.tile([1, 1], f32)
            nc.vector.reciprocal(rt_sb[:], t_sb[:])
            rt_dram = dram_pool.tile([1, 1], f32)
            nc.sync.dma_start(rt_dram[:], rt_sb[:])
            rt_b = cpool.tile([128, 1], f32)
            nc.sync.dma_start(rt_b[:], rt_dram[:].to_broadcast([128, 1]))

            # ---- Phase A: load Q/K for all pairs; row ssq on DVE.
            q_raw, k_raw = {}, {}
            for p in range(PAIRS):
                q_raw[p] = raw_pool.tile([128, T, D], f32, tag=f"qraw{p}",
                                         name=f"qraw{p}")
                nc.sync.dma_start(
                    q_raw[p][:],
                    q_hbm.ap()[p].rearrange("(t pp) d -> pp t d", pp=128))
                k_raw[p] = raw_pool.tile([128, T, D], f32, tag=f"kraw{p}",
                                         name=f"kraw{p}")
                nc.sync.dma_start(
                    k_raw[p][:],
                    k_hbm.ap()[p].rearrange("(t pp) d -> pp t d", pp=128))

            ssq = cpool.tile([128, 2 * PAIRS, T], f32, name="ssq")
            for p in range(PAIRS):
                for i, srct in ((0, q_raw[p]), (1, k_raw[p])):
                    sq = work_pool.tile([128, T, D], f32, tag="sq")
                    nc.vector.tensor_mul(sq[:], srct[:], srct[:])
                    nc.vector.tensor_reduce(
                        ssq[:, 2 * p + i, :], sq[:],
                        axis=mybir.AxisListType.X, op=mybir.AluOpType.add)

            # rsqrt = sqrt(1/x): DVE reciprocal then one ACT Sqrt for all
            # pairs, then force the Exp table to load before the main loop
            # (dummy exp on a tiny slice).
            inv_ssq = cpool.tile([128, 2 * PAIRS, T], f32, name="inv_ssq")
            nc.vector.reciprocal(inv_ssq[:], ssq[:])
            r_all = cpool.tile([128, 2 * PAIRS, T], f32, name="r_all")
            nc.scalar.activation(r_all[:], inv_ssq[:], AF.Sqrt)
            dummy = cpool.tile([128, 1], f32, name="dummy")
            nc.scalar.activation(dummy[:], ssq[:, 0, 0:1], AF.Exp)
            # Fold 1/temp into K's row scales.
            for p in range(PAIRS):
                nc.vector.tensor_scalar_mul(
                    r_all[:, 2 * p + 1, :], r_all[:, 2 * p + 1, :], rt_b[:])

            def bcast(rr, w):
                return bass.AP(tensor=rr.tensor, offset=rr.offset,
                               ap=[rr.ap[0], rr.ap[1], [0, w]])

            # ---- Phase B (per pair): normalize into bf16 staging.
            # qn2 holds each l-tile's normalized Q duplicated side by side so
            # one PE transpose lands it in both partition halves of qnT_dup.
            def phase_b(p):
                qn2 = norm_pool.tile([128, T, 2, D], bf16, tag="qn2")
                kn = norm_pool.tile([128, T, D], bf16, tag="kn")
                rq = r_all[:, 2 * p, :]
                rk = r_all[:, 2 * p + 1, :]
                nc.vector.tensor_mul(qn2[:, :, 0, :], q_raw[p][:], bcast(rq, D))
                nc.vector.tensor_copy(qn2[:, :, 1, :], qn2[:, :, 0, :])
                nc.vector.tensor_mul(kn[:], k_raw[p][:], bcast(rk, D))
                return qn2, kn

            # ---- Phase C (per pair): PE transposes -> qnT_dup / knT_pack.
            # Emitted as steps so pair p+1's transposes interleave into
            # main(p)'s PE stream.
            def phase_c_tiles(p):
                qnT = big_pool.tile([128, L], bf16, tag="qnT", name=f"qnT{p}")
                knT = big_pool.tile([128, TP, 128], bf16, tag="knT",
                                    name=f"knT{p}")
                return qnT, knT

            def phase_c_steps(p, qn2, kn, qnT, knT):
                steps = []
                for t in range(T):
                    def qstep(t=t):
                        tp = ptp_pool.tile([128, 128], bf16, tag="ptp")
                        nc.tensor.transpose(
                            tp[:], qn2[:, t, :, :].rearrange("p a b -> p (a b)"),
                            identity_bf[:])
                        nc.vector.tensor_copy(qnT[:, ds(t * 128, 128)], tp[:])
                    steps.append(qstep)
                for j in range(TP):
                    def kstep(j=j):
                        tp = ptp_pool.tile([128, 128], bf16, tag="ptp")
                        nc.tensor.transpose(
                            tp[:], kn[:, 2 * j:2 * j + 2, :].rearrange(
                                "p a b -> p (a b)"),
                            identity_bf[:])
                        nc.vector.tensor_copy(knT[:, j, :], tp[:])
                    steps.append(kstep)
                return steps

            # ---- V load + bf16 convert with ones column appended.
            def v_load(p):
                v_stage = io_pool.tile([128, T, D], f32, tag="vstage")
                nc.sync.dma_start(
                    v_stage[:],
                    v_hbm.ap()[p].rearrange("(t pp) d -> pp t d", pp=128))
                v_aug = io_pool.tile([128, T, D + 1], bf16, tag="vaug")
                nc.vector.memset(v_aug[:, :, D:D + 1], 1.0)
                nc.vector.tensor_copy(v_aug[:, :, 0:D], v_stage[:])
                return v_aug

            # ---- Main loop for one pair. `extra` holds interleave steps for
            # the next pair, popped a couple per (g, j) iteration.
            def main_loop(p, qnT, knT, v_aug, extra):
                for g in range(NG):
                    ps2 = psum2_pool.tile([D + 1, GCH, CW], f32, tag="ps2")
                    for j in range(TP):
                        ps1 = [None, None]
                        pex = [None, None]
                        for lc in range(GCH):
                            ps1[lc] = psum1_pool.tile([128, 2, CW], f32,
                                                      tag="ps1", name="ps1")
                        col0 = g * 1024
                        # mm1: even tile on rows 0-63 (tile_position (0,0)),
                        # odd on rows 64-127 ((64,0)); run concurrently.
                        for lc in range(GCH):
                            c = col0 + lc * CW
                            nc.tensor.matmul(
                                ps1[lc][:, 0, :], knT[0:64, j, :],
                                qnT[0:64, ds(c, CW)])
                            nc.tensor.matmul(
                                ps1[lc][:, 1, :], knT[64:128, j, :],
                                qnT[64:128, ds(c, CW)])
                        for lc in range(GCH):
                            pex[lc] = pexp_pool.tile([128, 2, CW], bf16,
                                                     tag="pexp", name="pex")
                            nc.scalar.activation(pex[lc][:], ps1[lc][:], AF.Exp)
                        # mm2: accumulate out^T; ones column -> denominator.
                        for par in range(2):
                            st = 2 * j + par
                            for lc in range(GCH):
                                nc.tensor.matmul(
                                    ps2[:, lc, :], v_aug[:, st, :],
                                    pex[lc][:, par, :],
                                    start=(j == 0 and par == 0),
                                    stop=(j == TP - 1 and par == 1))
                        for _ in range(2):
                            if extra:
                                extra.pop(0)()

                    # Epilogue for this L half.
                    o_sb = work_pool.tile([D + 1, GCH, CW], bf16, tag="osb")
                    nc.vector.tensor_copy(o_sb[:], ps2[:])
                    for lc in range(GCH):
                        for b in range(CW // 128):
                            tp = ptp_pool.tile([128, 128], bf16, tag="ptp")
                            nc.tensor.transpose(
                                tp[:, 0:D + 1],
                                o_sb[:, lc, ds(b * 128, 128)],
                                identity_bf[0:D + 1, 0:D + 1])
                            rcp = small_pool.tile([128, 1], f32, tag="rcp")
                            nc.vector.reciprocal(rcp[:], tp[:, D:D + 1])
                            o_fin = small_pool.tile([128, D], f32, tag="ofin")
                            nc.vector.tensor_scalar_mul(
                                o_fin[:], tp[:, 0:D], rcp[:])
                            nc.sync.dma_start(
                                o_hbm.ap()[p, ds(g * 1024 + lc * CW + b * 128,
                                                 128), :],
                                o_fin[:])
                            if extra:
                                extra.pop(0)()

            # ---- Drive all pairs with interleaving.
            qn2_0, kn_0 = phase_b(0)
            qnT_0, knT_0 = phase_c_tiles(0)
            for st in phase_c_steps(0, qn2_0, kn_0, qnT_0, knT_0):
                st()
            v_0 = v_load(0)

            cur = (qnT_0, knT_0, v_0)
            for p in range(PAIRS):
                extra = []
                nxt = None
                if p + 1 < PAIRS:
                    qn2_n, kn_n = phase_b(p + 1)
                    qnT_n, knT_n = phase_c_tiles(p + 1)
                    extra = phase_c_steps(p + 1, qn2_n, kn_n, qnT_n, knT_n)
                    vl = [None]

                    def vstep(pn=p + 1, vl=vl):
                        vl[0] = v_load(pn)
                    extra.insert(12, vstep)
                    nxt = (qnT_n, knT_n, vl)
                qnT, knT, v_aug = cur
                if isinstance(v_aug, list):
                    v_aug = v_aug[0]
                main_loop(p, qnT, knT, v_aug, extra)
                for st in extra:
                    st()
                if nxt is not None:
                    cur = nxt

    nc.compile()
    return nc


def _get_program():
    if "nc" not in _PROGRAM_CACHE:
        _PROGRAM_CACHE["nc"] = _build_program()
    return _PROGRAM_CACHE["nc"]


def kernel(queries, keys, values, temp_scale):
    from concourse.bass_utils import run_bass_kernel_spmd

    N, Lq, H, Dh = queries.shape
    assert (N, Lq, H, Dh) == (4, L, 8, D), (N, Lq, H, Dh)

    # [N, L, H, D] -> [N*H, L, D]; core c owns pairs 4c..4c+4.
    def shard(x):
        x = np.ascontiguousarray(
            np.asarray(x, dtype=np.float32).transpose(0, 2, 1, 3)
        ).reshape(N * H, Lq, Dh)
        return [np.ascontiguousarray(x[PAIRS * c:PAIRS * (c + 1)])
                for c in range(N_CORES)]

    qs, ks, vs = shard(queries), shard(keys), shard(values)
    t11 = np.asarray(temp_scale, dtype=np.float32).reshape(1, 1)
    in_maps = [
        {"q": qs[c], "k": ks[c], "v": vs[c], "temp": t11}
        for c in range(N_CORES)
    ]

    nc = _get_program()
    res = run_bass_kernel_spmd(nc, in_maps, core_ids=list(range(N_CORES)))
    if getattr(res, "exec_time_ns", None):
        print(f"HW exec time: {res.exec_time_ns} ns")

    out = np.stack([res.results[c]["o"] for c in range(N_CORES)])  # [8,4,L,D]
    out = out.reshape(N, H, Lq, Dh).transpose(0, 2, 1, 3)          # [N,L,H,D]
    return np.ascontiguousarray(out)


# revision 10
# speedup vs baseline: 1.1917x; 1.1917x over previous
"""Cosine attention kernel for Trainium2, sharded over 8 NeuronCores.

Problem: N=4, L=S=2048, H=8, D=64 fp32.
  q = queries / ||queries||_D ; k = keys / ||keys||_D
  qk = einsum('nlhd,nshd->nlsh', q, k); A = softmax(qk / temp, axis=S)
  out = einsum('nlsh,nshd->nlhd', A, values)

Sharding: the 32 (n, h) pairs are split 4-per-core (data + head parallel).
Each core computes 4 independent 2048x2048 attention problems.

v2 design (ACT-bound target):
  - All matmul operands bf16. Row norms: DVE squares+reduce, one ACT Rsqrt
    for all pairs, exp table preloaded right after.
  - QK^T uses PE row-group tiling: knT packed as [128, j, 128] with even
    s-tile's D-rows on partitions 0-63 and odd s-tile's on 64-127; qnT is
    duplicated into both partition halves. Two K=64 matmuls with
    tile_position (0,0)/(64,0) run concurrently on the array.
  - Main loop (per pair, per L-half g, per s-tile-pair j): mm1 e/o for two
    512-col chunks reusing resident weights, one Exp per [128,2,512] PSUM
    tile -> bf16 SBUF, mm2 [128,65]x[128,512] accumulating out^T (ones
    column of V accumulates the softmax denominator in row 64).
  - Epilogue per (pair, g): copy ps2 -> bf16, PE-transpose [65,128] blocks,
    DVE reciprocal + scale, DMA out.
  - Transposes/normalize of pair p+1 are interleaved into main(p)'s issue
    stream so the ACT engine never waits at pair transitions.
"""

import sys

if "/opt/trn_rl_repo" not in sys.path:
    sys.path.insert(0, "/opt/trn_rl_repo")

import numpy as np

N_CORES = 8
PAIRS = 4          # (n, h) pairs per core
L = 2048           # query length
S = 2048           # key length
D = 64             # head dim
T = S // 128       # 128-row s-tiles per pair (16)
TP = T // 2        # s-tile pairs (8)
NG = 2             # L halves
GCH = 2            # 512-col chunks per L half
CW = 512           # chunk width

_PROGRAM_CACHE = {}


def _build_program():
    import concourse.tile as tile
    import concourse.bass as bass
    from concourse import bacc, mybir
    from concourse.bass import ds
    from concourse.masks import make_identity

    f32 = mybir.dt.float32
    f32r = mybir.dt.float32r
    bf16 = mybir.dt.bfloat16
    AF = mybir.ActivationFunctionType

    nc = bacc.Bacc("TRN2", target_bir_lowering=False, debug=False,
                   num_devices=N_CORES)
    q_hbm = nc.dram_tensor("q", [PAIRS, L, D], f32, kind="ExternalInput")
    k_hbm = nc.dram_tensor("k", [PAIRS, S, D], f32, kind="ExternalInput")
    v_hbm = nc.dram_tensor("v", [PAIRS, S, D], f32, kind="ExternalInput")
    t_hbm = nc.dram_tensor("temp", [1, 1], f32, kind="ExternalInput")
    o_hbm = nc.dram_tensor("o", [PAIRS, L, D], f32, kind="ExternalOutput")

    with tile.TileContext(nc) as tc:
        with (
            tc.tile_pool(name="const", bufs=1) as cpool,
            tc.tile_pool(name="raw", bufs=1) as raw_pool,
            tc.tile_pool(name="norm", bufs=2) as norm_pool,
            tc.tile_pool(name="big", bufs=2) as big_pool,
            tc.tile_pool(name="io", bufs=2) as io_pool,
            tc.tile_pool(name="work", bufs=2) as work_pool,
            tc.tile_pool(name="small", bufs=4) as small_pool,
            tc.tile_pool(name="pexp", bufs=4) as pexp_pool,
            tc.tile_pool(name="psum1", bufs=2, space="PSUM") as psum1_pool,
            tc.tile_pool(name="psum2", bufs=1, space="PSUM") as psum2_pool,
            tc.tile_pool(name="ptp", bufs=2, space="PSUM") as ptp_pool,
            tc.tile_pool(name="dram", bufs=1, space="DRAM") as dram_pool,
        ):
            identity = cpool.tile([128, 128], f32)
            make_identity(nc, identity[:])
            identity_bf = cpool.tile([128, 128], bf16)
            nc.vector.tensor_copy(identity_bf[:], identity[:])

            scratch_f = cpool.tile([128, 512], f32)
            nc.vector.memset(scratch_f[:], 0.0)
            scratch_bf = cpool.tile([128, 512], bf16)
            nc.vector.tensor_copy(scratch_bf[:], scratch_f[:])

            def warm(n):
                # N=512 bf16 regular matmuls: HAM activity to keep the PE
                # p-state ramped while DMAs stream in.
                for _ in range(n):
                    wk = psum1_pool.tile([128, 2, CW], f32, tag="ps1", name="wk")
                    nc.tensor.matmul(wk[:, 0, :], identity_bf[:], scratch_bf[:])

            warm(10)

            # 1/temp broadcast to [128, 1] (bounce through DRAM for the
            # partition-broadcast DMA).
            t_sb = cpool.tile([1, 1], f32)
            nc.sync.dma_start(t_sb[:], t_hbm.ap())
            rt_sb = cpool.tile([1, 1], f32)
            nc.vector.reciprocal(rt_sb[:], t_sb[:])
            rt_dram = dram_pool.tile([1, 1], f32)
            nc.sync.dma_start(rt_dram[:], rt_sb[:])
            rt_b = cpool.tile([128, 1], f32)
            nc.sync.dma_start(rt_b[:], rt_dram[:].to_broadcast([128, 1]))

            # ---- Phase A: load Q/K, pair 0 first so its pipeline starts
            # while pairs 1-3 stream in.
            q_raw, k_raw = {}, {}
            for p in range(PAIRS):
                q_raw[p] = raw_pool.tile([128, T, D], f32, tag=f"qraw{p}",
                                         name=f"qraw{p}")
                nc.sync.dma_start(
                    q_raw[p][:],
                    q_hbm.ap()[p].rearrange("(t pp) d -> pp t d", pp=128))
                k_raw[p] = raw_pool.tile([128, T, D], f32, tag=f"kraw{p}",
                                         name=f"kraw{p}")
                nc.sync.dma_start(
                    k_raw[p][:],
                    k_hbm.ap()[p].rearrange("(t pp) d -> pp t d", pp=128))

            # Preload the Sqrt activation table while DMAs stream.
            dummy = cpool.tile([128, 1], f32, name="dummy")
            nc.scalar.activation(dummy[:], scratch_f[:, 0:1], AF.Sqrt)

            ssq = cpool.tile([128, 2 * PAIRS, T], f32, name="ssq")
            inv_ssq = cpool.tile([128, 2 * PAIRS, T], f32, name="inv_ssq")
            r_all = cpool.tile([128, 2 * PAIRS, T], f32, name="r_all")
            dummy2 = cpool.tile([128, 1], f32, name="dummy2")

            def ssq_steps(p):
                for i, srct in ((0, q_raw[p]), (1, k_raw[p])):
                    sq = work_pool.tile([128, T, D], f32, tag="sq")
                    nc.vector.tensor_mul(sq[:], srct[:], srct[:])
                    nc.vector.tensor_reduce(
                        ssq[:, 2 * p + i, :], sq[:],
                        axis=mybir.AxisListType.X, op=mybir.AluOpType.add)

            def rsqrt_group(sl):
                # rsqrt = sqrt(1/x): DVE reciprocal + ACT Sqrt.
                nc.vector.reciprocal(inv_ssq[:, sl, :], ssq[:, sl, :])
                nc.scalar.activation(r_all[:, sl, :], inv_ssq[:, sl, :],
                                     AF.Sqrt)

            # Pair 0 fast path: ssq -> rsqrt -> exp-table preload.
            ssq_steps(0)
            rsqrt_group(slice(0, 2))
            nc.scalar.activation(dummy2[:], ssq[:, 0, 0:1], AF.Exp)
            # Fold 1/temp into K's row scales (per pair, after its rsqrt).
            def rk_scale(p):
                nc.vector.tensor_scalar_mul(
                    r_all[:, 2 * p + 1, :], r_all[:, 2 * p + 1, :], rt_b[:])
            rk_scale(0)

            def bcast(rr, w):
                return bass.AP(tensor=rr.tensor, offset=rr.offset,
                               ap=[rr.ap[0], rr.ap[1], [0, w]])

            # ---- Phase B (per pair): normalize into bf16 staging.
            # qn2 holds each l-tile's normalized Q duplicated side by side so
            # one PE transpose lands it in both partition halves of qnT_dup.
            def phase_b(p):
                qn2 = norm_pool.tile([128, T, 2, D], bf16, tag="qn2")
                kn = norm_pool.tile([128, T, D], bf16, tag="kn")
                rq = r_all[:, 2 * p, :]
                rk = r_all[:, 2 * p + 1, :]
                nc.vector.tensor_mul(qn2[:, :, 0, :], q_raw[p][:], bcast(rq, D))
                nc.vector.tensor_copy(qn2[:, :, 1, :], qn2[:, :, 0, :])
                nc.vector.tensor_mul(kn[:], k_raw[p][:], bcast(rk, D))
                return qn2, kn

            # ---- Phase C (per pair): PE transposes -> qnT_dup / knT_pack.
            # Emitted as steps so pair p+1's transposes interleave into
            # main(p)'s PE stream.
            def phase_c_tiles(p):
                qnT = big_pool.tile([128, L], bf16, tag="qnT", name=f"qnT{p}")
                knT = big_pool.tile([128, TP, 128], bf16, tag="knT",
                                    name=f"knT{p}")
                return qnT, knT

            def phase_c_steps(p, qn2, kn, qnT, knT):
                steps = []
                for t in range(T):
                    def qstep(t=t):
                        tp = ptp_pool.tile([128, 128], bf16, tag="ptp")
                        nc.tensor.transpose(
                            tp[:], qn2[:, t, :, :].rearrange("p a b -> p (a b)"),
                            identity_bf[:])
                        nc.vector.tensor_copy(qnT[:, ds(t * 128, 128)], tp[:])
                    steps.append(qstep)
                for j in range(TP):
                    def kstep(j=j):
                        tp = ptp_pool.tile([128, 128], bf16, tag="ptp")
                        nc.tensor.transpose(
                            tp[:], kn[:, 2 * j:2 * j + 2, :].rearrange(
                                "p a b -> p (a b)"),
                            identity_bf[:])
                        nc.vector.tensor_copy(knT[:, j, :], tp[:])
                    steps.append(kstep)
                return steps

            # ---- V load + f32r convert with ones column appended.
            def v_load(p):
                v_stage = io_pool.tile([128, T, D + 1], f32, tag="vstage")
                nc.vector.memset(v_stage[:, :, D:D + 1], 1.0)
                nc.sync.dma_start(
                    v_stage[:, :, 0:D],
                    v_hbm.ap()[p].rearrange("(t pp) d -> pp t d", pp=128))
                v_aug = io_pool.tile([128, T, D + 1], f32r, tag="vaug")
                nc.vector.tensor_copy(v_aug[:], v_stage[:])
                return v_aug

            # ---- Main loop for one pair. `extra` holds interleave steps for
            # the next pair, popped a couple per (g, j) iteration.
            def main_loop(p, qnT, knT, v_aug, extra):
                for g in range(NG):
                    ps2 = psum2_pool.tile([D + 1, GCH, CW], f32, tag="ps2")
                    for j in range(TP):
                        ps1 = [None, None]
                        pex = [None, None]
                        for lc in range(GCH):
                            ps1[lc] = psum1_pool.tile([128, 2, CW], f32,
                                                      tag="ps1", name="ps1")
                        col0 = g * 1024
                        # mm1: even tile on rows 0-63 (tile_position (0,0)),
                        # odd on rows 64-127 ((64,0)); run concurrently.
                        for lc in range(GCH):
                            c = col0 + lc * CW
                            nc.tensor.matmul(
                                ps1[lc][:, 0, :], knT[0:64, j, :],
                                qnT[0:64, ds(c, CW)])
                            nc.tensor.matmul(
                                ps1[lc][:, 1, :], knT[64:128, j, :],
                                qnT[64:128, ds(c, CW)])
                        for lc in range(GCH):
                            pex[lc] = pexp_pool.tile([128, 2, CW], f32r,
                                                     tag="pexp", name="pex")
                            nc.scalar.activation(
                                pex[lc][:].rearrange("p a b -> p (a b)"),
                                ps1[lc][:].rearrange("p a b -> p (a b)"),
                                AF.Exp)
                        # mm2: accumulate out^T; ones column -> denominator.
                        for par in range(2):
                            st = 2 * j + par
                            for lc in range(GCH):
                                nc.tensor.matmul(
                                    ps2[:, lc, :], v_aug[:, st, :],
                                    pex[lc][:, par, :],
                                    start=(j == 0 and par == 0),
                                    stop=(j == TP - 1 and par == 1))
                        for _ in range(2):
                            if extra:
                                extra.pop(0)()

                    # Epilogue for this L half.
                    o_sb = work_pool.tile([D + 1, GCH, CW], bf16, tag="osb")
                    nc.vector.tensor_copy(o_sb[:], ps2[:])
                    for lc in range(GCH):
                        for b in range(CW // 128):
                            tp = ptp_pool.tile([128, 128], bf16, tag="ptp")
                            nc.tensor.transpose(
                                tp[:, 0:D + 1],
                                o_sb[:, lc, ds(b * 128, 128)],
                                identity_bf[0:D + 1, 0:D + 1])
                            rcp = small_pool.tile([128, 1], f32, tag="rcp")
                            nc.vector.reciprocal(rcp[:], tp[:, D:D + 1])
                            o_fin = small_pool.tile([128, D], f32, tag="ofin")
                            nc.vector.tensor_scalar_mul(
                                o_fin[:], tp[:, 0:D], rcp[:])
                            nc.sync.dma_start(
                                o_hbm.ap()[p, ds(g * 1024 + lc * CW + b * 128,
                                                 128), :],
                                o_fin[:])
                            if extra:
                                extra.pop(0)()

            # ---- Drive all pairs with interleaving.
            qn2_0, kn_0 = phase_b(0)
            qnT_0, knT_0 = phase_c_tiles(0)
            for st in phase_c_steps(0, qn2_0, kn_0, qnT_0, knT_0):
                st()
            v_0 = v_load(0)

            # Pairs 1-3 row norms: DVE work runs behind pair 0's chain; the
            # one grouped Sqrt costs a single Sqrt+Exp table reload early in
            # main(0).
            for p in range(1, PAIRS):
                ssq_steps(p)
            rsqrt_group(slice(2, 2 * PAIRS))
            for p in range(1, PAIRS):
                rk_scale(p)

            cur = (qnT_0, knT_0, v_0)
            for p in range(PAIRS):
                extra = []
                nxt = None
                if p + 1 < PAIRS:
                    qn2_n, kn_n = phase_b(p + 1)
                    qnT_n, knT_n = phase_c_tiles(p + 1)
                    extra = phase_c_steps(p + 1, qn2_n, kn_n, qnT_n, knT_n)
                    vl = [None]

                    def vstep(pn=p + 1, vl=vl):
                        vl[0] = v_load(pn)
                    extra.insert(12, vstep)
                    nxt = (qnT_n, knT_n, vl)
                qnT, knT, v_aug = cur
                if isinstance(v_aug, list):
                    v_aug = v_aug[0]
                main_loop(p, qnT, knT, v_aug, extra)
                for st in extra:
                    st()
                if nxt is not None:
                    cur = nxt

    nc.compile()
    return nc


def _get_program():
    if "nc" not in _PROGRAM_CACHE:
        _PROGRAM_CACHE["nc"] = _build_program()
    return _PROGRAM_CACHE["nc"]


def kernel(queries, keys, values, temp_scale):
    from concourse.bass_utils import run_bass_kernel_spmd

    N, Lq, H, Dh = queries.shape
    assert (N, Lq, H, Dh) == (4, L, 8, D), (N, Lq, H, Dh)

    # [N, L, H, D] -> [N*H, L, D]; core c owns pairs 4c..4c+4.
    def shard(x):
        x = np.ascontiguousarray(
            np.asarray(x, dtype=np.float32).transpose(0, 2, 1, 3)
        ).reshape(N * H, Lq, Dh)
        return [np.ascontiguousarray(x[PAIRS * c:PAIRS * (c + 1)])
                for c in range(N_CORES)]

    qs, ks, vs = shard(queries), shard(keys), shard(values)
    t11 = np.asarray(temp_scale, dtype=np.float32).reshape(1, 1)
    in_maps = [
        {"q": qs[c], "k": ks[c], "v": vs[c], "temp": t11}
        for c in range(N_CORES)
    ]

    nc = _get_program()
    res = run_bass_kernel_spmd(nc, in_maps, core_ids=list(range(N_CORES)))
    if getattr(res, "exec_time_ns", None):
        print(f"HW exec time: {res.exec_time_ns} ns")

    out = np.stack([res.results[c]["o"] for c in range(N_CORES)])  # [8,4,L,D]
    out = out.reshape(N, H, Lq, Dh).transpose(0, 2, 1, 3)          # [N,L,H,D]
    return np.ascontiguousarray(out)


# revision 13
# speedup vs baseline: 1.2195x; 1.0233x over previous
"""Cosine attention kernel for Trainium2, sharded over 8 NeuronCores.

Problem: N=4, L=S=2048, H=8, D=64 fp32.
  q = queries / ||queries||_D ; k = keys / ||keys||_D
  qk = einsum('nlhd,nshd->nlsh', q, k); A = softmax(qk / temp, axis=S)
  out = einsum('nlsh,nshd->nlhd', A, values)

Sharding: the 32 (n, h) pairs are split 4-per-core (data + head parallel).
Each core computes 4 independent 2048x2048 attention problems.

v2 design (ACT-bound target):
  - All matmul operands bf16. Row norms: DVE squares+reduce, one ACT Rsqrt
    for all pairs, exp table preloaded right after.
  - QK^T uses PE row-group tiling: knT packed as [128, j, 128] with even
    s-tile's D-rows on partitions 0-63 and odd s-tile's on 64-127; qnT is
    duplicated into both partition halves. Two K=64 matmuls with
    tile_position (0,0)/(64,0) run concurrently on the array.
  - Main loop (per pair, per L-half g, per s-tile-pair j): mm1 e/o for two
    512-col chunks reusing resident weights, one Exp per [128,2,512] PSUM
    tile -> bf16 SBUF, mm2 [128,65]x[128,512] accumulating out^T (ones
    column of V accumulates the softmax denominator in row 64).
  - Epilogue per (pair, g): copy ps2 -> bf16, PE-transpose [65,128] blocks,
    DVE reciprocal + scale, DMA out.
  - Transposes/normalize of pair p+1 are interleaved into main(p)'s issue
    stream so the ACT engine never waits at pair transitions.
"""

import sys

if "/opt/trn_rl_repo" not in sys.path:
    sys.path.insert(0, "/opt/trn_rl_repo")

import numpy as np

N_CORES = 8
PAIRS = 4          # (n, h) pairs per core
L = 2048           # query length
S = 2048           # key length
D = 64             # head dim
T = S // 128       # 128-row s-tiles per pair (16)
TP = T // 2        # s-tile pairs (8)
NG = 2             # L halves
GCH = 2            # 512-col chunks per L half
CW = 512           # chunk width

_PROGRAM_CACHE = {}


def _build_program():
    import concourse.tile as tile
    import concourse.bass as bass
    from concourse import bacc, mybir
    from concourse.bass import ds
    from concourse.masks import make_identity

    f32 = mybir.dt.float32
    f32r = mybir.dt.float32r
    bf16 = mybir.dt.bfloat16
    AF = mybir.ActivationFunctionType

    nc = bacc.Bacc("TRN2", target_bir_lowering=False, debug=False,
                   num_devices=N_CORES)
    q_hbm = nc.dram_tensor("q", [PAIRS, L, D], f32, kind="ExternalInput")
    k_hbm = nc.dram_tensor("k", [PAIRS, S, D], f32, kind="ExternalInput")
    v_hbm = nc.dram_tensor("v", [PAIRS, S, D], f32, kind="ExternalInput")
    t_hbm = nc.dram_tensor("temp", [1, 1], f32, kind="ExternalInput")
    o_hbm = nc.dram_tensor("o", [PAIRS, L, D], f32, kind="ExternalOutput")

    with tile.TileContext(nc) as tc:
        with (
            tc.tile_pool(name="const", bufs=1) as cpool,
            tc.tile_pool(name="raw", bufs=1) as raw_pool,
            tc.tile_pool(name="norm", bufs=2) as norm_pool,
            tc.tile_pool(name="big", bufs=2) as big_pool,
            tc.tile_pool(name="io", bufs=2) as io_pool,
            tc.tile_pool(name="work", bufs=2) as work_pool,
            tc.tile_pool(name="small", bufs=4) as small_pool,
            tc.tile_pool(name="pexp", bufs=6) as pexp_pool,
            tc.tile_pool(name="psum1", bufs=2, space="PSUM") as psum1_pool,
            tc.tile_pool(name="psum2", bufs=1, space="PSUM") as psum2_pool,
            tc.tile_pool(name="ptp", bufs=2, space="PSUM") as ptp_pool,
            tc.tile_pool(name="dram", bufs=1, space="DRAM") as dram_pool,
        ):
            identity = cpool.tile([128, 128], f32)
            make_identity(nc, identity[:])
            identity_bf = cpool.tile([128, 128], bf16)
            nc.vector.tensor_copy(identity_bf[:], identity[:])

            scratch_f = cpool.tile([128, 512], f32)
            nc.vector.memset(scratch_f[:], 0.0)
            scratch_bf = cpool.tile([128, 512], bf16)
            nc.vector.tensor_copy(scratch_bf[:], scratch_f[:])

            def warm(n):
                # N=512 bf16 regular matmuls: HAM activity to keep the PE
                # p-state ramped while DMAs stream in.
                for _ in range(n):
                    wk = psum1_pool.tile([128, 2, CW], f32, tag="ps1", name="wk")
                    nc.tensor.matmul(wk[:, 0, :], identity_bf[:], scratch_bf[:])

            # ---- Phase A: load Q/K, pair 0 first (descriptor generation is
            # serial on SP, so pair 0's descriptors lead every DMA ring).
            q_raw, k_raw = {}, {}
            for p in range(PAIRS):
                q_raw[p] = raw_pool.tile([128, T, D], f32, tag=f"qraw{p}",
                                         name=f"qraw{p}")
                k_raw[p] = raw_pool.tile([128, T, D], f32, tag=f"kraw{p}",
                                         name=f"kraw{p}")

            def qk_dma(p):
                nc.sync.dma_start(
                    q_raw[p][:],
                    q_hbm.ap()[p].rearrange("(t pp) d -> pp t d", pp=128))
                nc.sync.dma_start(
                    k_raw[p][:],
                    k_hbm.ap()[p].rearrange("(t pp) d -> pp t d", pp=128))

            qk_dma(0)

            # 1/temp broadcast to [128, 1] (bounce through DRAM for the
            # partition-broadcast DMA).
            t_sb = cpool.tile([1, 1], f32)
            nc.sync.dma_start(t_sb[:], t_hbm.ap())
            rt_sb = cpool.tile([1, 1], f32)
            nc.vector.reciprocal(rt_sb[:], t_sb[:])
            rt_dram = dram_pool.tile([1, 1], f32)
            nc.sync.dma_start(rt_dram[:], rt_sb[:])
            rt_b = cpool.tile([128, 1], f32)
            nc.sync.dma_start(rt_b[:], rt_dram[:].to_broadcast([128, 1]))

            warm(10)
            for p in range(1, PAIRS):
                qk_dma(p)

            # Preload the Sqrt activation table while DMAs stream.
            dummy = cpool.tile([128, 1], f32, name="dummy")
            nc.scalar.activation(dummy[:], scratch_f[:, 0:1], AF.Sqrt)

            ssq = cpool.tile([128, 2 * PAIRS, T], f32, name="ssq")
            inv_ssq = cpool.tile([128, 2 * PAIRS, T], f32, name="inv_ssq")
            r_all = cpool.tile([128, 2 * PAIRS, T], f32, name="r_all")
            dummy2 = cpool.tile([128, 1], f32, name="dummy2")

            def ssq_steps(p):
                for i, srct in ((0, q_raw[p]), (1, k_raw[p])):
                    sq = work_pool.tile([128, T, D], f32, tag="sq")
                    nc.vector.tensor_mul(sq[:], srct[:], srct[:])
                    nc.vector.tensor_reduce(
                        ssq[:, 2 * p + i, :], sq[:],
                        axis=mybir.AxisListType.X, op=mybir.AluOpType.add)

            def rsqrt_group(sl):
                # rsqrt = sqrt(1/x): DVE reciprocal + ACT Sqrt.
                nc.vector.reciprocal(inv_ssq[:, sl, :], ssq[:, sl, :])
                nc.scalar.activation(r_all[:, sl, :], inv_ssq[:, sl, :],
                                     AF.Sqrt)

            # Pair 0 fast path: ssq -> rsqrt -> exp-table preload.
            ssq_steps(0)
            rsqrt_group(slice(0, 2))
            nc.scalar.activation(dummy2[:], ssq[:, 0, 0:1], AF.Exp)
            # Fold 1/temp into K's row scales (per pair, after its rsqrt).
            def rk_scale(p):
                nc.vector.tensor_scalar_mul(
                    r_all[:, 2 * p + 1, :], r_all[:, 2 * p + 1, :], rt_b[:])
            rk_scale(0)

            def bcast(rr, w):
                return bass.AP(tensor=rr.tensor, offset=rr.offset,
                               ap=[rr.ap[0], rr.ap[1], [0, w]])

            # ---- Phase B (per pair): normalize into bf16 staging.
            # qn2 holds each l-tile's normalized Q duplicated side by side so
            # one PE transpose lands it in both partition halves of qnT_dup.
            def phase_b(p):
                qn2 = norm_pool.tile([128, T, 2, D], bf16, tag="qn2")
                kn = norm_pool.tile([128, T, D], bf16, tag="kn")
                rq = r_all[:, 2 * p, :]
                rk = r_all[:, 2 * p + 1, :]
                nc.vector.tensor_mul(qn2[:, :, 0, :], q_raw[p][:], bcast(rq, D))
                nc.vector.tensor_copy(qn2[:, :, 1, :], qn2[:, :, 0, :])
                nc.vector.tensor_mul(kn[:], k_raw[p][:], bcast(rk, D))
                return qn2, kn

            # ---- Phase C (per pair): PE transposes -> qnT_dup / knT_pack.
            # Emitted as steps so pair p+1's transposes interleave into
            # main(p)'s PE stream.
            def phase_c_tiles(p):
                qnT = big_pool.tile([128, L], bf16, tag="qnT", name=f"qnT{p}")
                knT = big_pool.tile([128, TP, 128], bf16, tag="knT",
                                    name=f"knT{p}")
                return qnT, knT

            def phase_c_steps(p, qn2, kn, qnT, knT):
                def qstep(t):
                    def run():
                        tp = ptp_pool.tile([128, 128], bf16, tag="ptp")
                        nc.tensor.transpose(
                            tp[:], qn2[:, t, :, :].rearrange("p a b -> p (a b)"),
                            identity_bf[:])
                        nc.vector.tensor_copy(qnT[:, ds(t * 128, 128)], tp[:])
                    return run

                def kstep(j):
                    def run():
                        tp = ptp_pool.tile([128, 128], bf16, tag="ptp")
                        nc.tensor.transpose(
                            tp[:], kn[:, 2 * j:2 * j + 2, :].rearrange(
                                "p a b -> p (a b)"),
                            identity_bf[:])
                        nc.vector.tensor_copy(knT[:, j, :], tp[:])
                    return run

                # Order so main(p)'s first iteration (k-pair 0, q cols of the
                # first L half) unblocks after just a few steps.
                steps = [kstep(0)]
                steps += [qstep(t) for t in range(8)]
                steps += [kstep(j) for j in range(1, TP)]
                steps += [qstep(t) for t in range(8, T)]
                return steps

            # ---- V load + f32r convert with ones column appended.
            def v_load(p):
                v_stage = io_pool.tile([128, T, D + 1], f32, tag="vstage")
                nc.vector.memset(v_stage[:, :, D:D + 1], 1.0)
                nc.sync.dma_start(
                    v_stage[:, :, 0:D],
                    v_hbm.ap()[p].rearrange("(t pp) d -> pp t d", pp=128))
                v_aug = io_pool.tile([128, T, D + 1], f32r, tag="vaug")
                nc.vector.tensor_copy(v_aug[:], v_stage[:])
                return v_aug

            # ---- Main loop for one pair. `extra` holds interleave steps for
            # the next pair, popped a couple per (g, j) iteration.
            def main_loop(p, qnT, knT, v_aug, extra):
                for g in range(NG):
                    ps2 = psum2_pool.tile([D + 1, GCH, CW], f32, tag="ps2")
                    for j in range(TP):
                        ps1 = [None, None]
                        pex = [None, None]
                        for lc in range(GCH):
                            ps1[lc] = psum1_pool.tile([128, 2, CW], f32,
                                                      tag="ps1", name="ps1")
                        col0 = g * 1024
                        # mm1: even tile on rows 0-63 (tile_position (0,0)),
                        # odd on rows 64-127 ((64,0)); run concurrently.
                        for lc in range(GCH):
                            c = col0 + lc * CW
                            nc.tensor.matmul(
                                ps1[lc][:, 0, :], knT[0:64, j, :],
                                qnT[0:64, ds(c, CW)])
                            nc.tensor.matmul(
                                ps1[lc][:, 1, :], knT[64:128, j, :],
                                qnT[64:128, ds(c, CW)])
                        for lc in range(GCH):
                            pex[lc] = pexp_pool.tile([128, 2, CW], f32r,
                                                     tag="pexp", name="pex")
                            nc.scalar.activation(
                                pex[lc][:].rearrange("p a b -> p (a b)"),
                                ps1[lc][:].rearrange("p a b -> p (a b)"),
                                AF.Exp)
                        # mm2: accumulate out^T; ones column -> denominator.
                        for par in range(2):
                            st = 2 * j + par
                            for lc in range(GCH):
                                nc.tensor.matmul(
                                    ps2[:, lc, :], v_aug[:, st, :],
                                    pex[lc][:, par, :],
                                    start=(j == 0 and par == 0),
                                    stop=(j == TP - 1 and par == 1))
                        for _ in range(2):
                            if extra:
                                extra.pop(0)()

                    # Epilogue for this L half.
                    o_sb = work_pool.tile([D + 1, GCH, CW], bf16, tag="osb")
                    nc.vector.tensor_copy(o_sb[:], ps2[:])
                    for lc in range(GCH):
                        for b in range(CW // 128):
                            tp = ptp_pool.tile([128, 128], bf16, tag="ptp")
                            nc.tensor.transpose(
                                tp[:, 0:D + 1],
                                o_sb[:, lc, ds(b * 128, 128)],
                                identity_bf[0:D + 1, 0:D + 1])
                            rcp = small_pool.tile([128, 1], f32, tag="rcp")
                            nc.vector.reciprocal(rcp[:], tp[:, D:D + 1])
                            o_fin = small_pool.tile([128, D], f32, tag="ofin")
                            nc.vector.tensor_scalar_mul(
                                o_fin[:], tp[:, 0:D], rcp[:])
                            nc.sync.dma_start(
                                o_hbm.ap()[p, ds(g * 1024 + lc * CW + b * 128,
                                                 128), :],
                                o_fin[:])
                            if extra:
                                extra.pop(0)()

            # ---- Drive all pairs with interleaving.
            qn2_0, kn_0 = phase_b(0)
            qnT_0, knT_0 = phase_c_tiles(0)
            for st in phase_c_steps(0, qn2_0, kn_0, qnT_0, knT_0):
                st()
            v_0 = v_load(0)

            # Pairs 1-3 row norms: DVE work runs behind pair 0's chain; the
            # one grouped Sqrt costs a single Sqrt+Exp table reload early in
            # main(0).
            for p in range(1, PAIRS):
                ssq_steps(p)
            rsqrt_group(slice(2, 2 * PAIRS))
            for p in range(1, PAIRS):
                rk_scale(p)

            cur = (qnT_0, knT_0, v_0)
            for p in range(PAIRS):
                extra = []
                nxt = None
                if p + 1 < PAIRS:
                    qn2_n, kn_n = phase_b(p + 1)
                    qnT_n, knT_n = phase_c_tiles(p + 1)
                    extra = phase_c_steps(p + 1, qn2_n, kn_n, qnT_n, knT_n)
                    vl = [None]

                    def vstep(pn=p + 1, vl=vl):
                        vl[0] = v_load(pn)
                    extra.insert(12, vstep)
                    nxt = (qnT_n, knT_n, vl)
                qnT, knT, v_aug = cur
                if isinstance(v_aug, list):
                    v_aug = v_aug[0]
                main_loop(p, qnT, knT, v_aug, extra)
                for st in extra:
                    st()
                if nxt is not None:
                    cur = nxt

    nc.compile()
    return nc


def _get_program():
    if "nc" not in _PROGRAM_CACHE:
        _PROGRAM_CACHE["nc"] = _build_program()
    return _PROGRAM_CACHE["nc"]


def kernel(queries, keys, values, temp_scale):
    from concourse.bass_utils import run_bass_kernel_spmd

    N, Lq, H, Dh = queries.shape
    assert (N, Lq, H, Dh) == (4, L, 8, D), (N, Lq, H, Dh)

    # [N, L, H, D] -> [N*H, L, D]; core c owns pairs 4c..4c+4.
    def shard(x):
        x = np.ascontiguousarray(
            np.asarray(x, dtype=np.float32).transpose(0, 2, 1, 3)
        ).reshape(N * H, Lq, Dh)
        return [np.ascontiguousarray(x[PAIRS * c:PAIRS * (c + 1)])
                for c in range(N_CORES)]

    qs, ks, vs = shard(queries), shard(keys), shard(values)
    t11 = np.asarray(temp_scale, dtype=np.float32).reshape(1, 1)
    in_maps = [
        {"q": qs[c], "k": ks[c], "v": vs[c], "temp": t11}
        for c in range(N_CORES)
    ]

    nc = _get_program()
    res = run_bass_kernel_spmd(nc, in_maps, core_ids=list(range(N_CORES)))
    if getattr(res, "exec_time_ns", None):
        print(f"HW exec time: {res.exec_time_ns} ns")

    out = np.stack([res.results[c]["o"] for c in range(N_CORES)])  # [8,4,L,D]
    out = out.reshape(N, H, Lq, Dh).transpose(0, 2, 1, 3)          # [N,L,H,D]
    return np.ascontiguousarray(out)
